# revision 1
# baseline (speedup 1.0000x reference)
"""Chamfer loss kernel for Trainium2 (Bass/Tile), 8 NeuronCores.

Math: for each batch b, D_b[n, m] = ||pred[b,n] - label[b,m]||.
result = mean_n(min_m D) + mean_m(min_n D).

Strategy
--------
Sharding: 8 cores = 4 batches x 2 halves of the pred axis. Core c
(b = c//2, h = c%2) owns queries q = pred[b, h*4096:(h+1)*4096] (NQ=4096)
and all refs r = label[b] (NR=8192). Each core makes ONE pass over its
4096 x 8192 block of the (negated) squared-distance matrix and produces
BOTH reductions from that single pass:
  - per-ref  max of -d^2 over its 4096 queries -> partial min_n; the two
    halves of a batch are combined on the host (tiny elementwise max).
  - per-query max of -d^2 over all 8192 refs -> complete min_m for its
    4096 pred points (finished on device: relu, sqrt, partial sums).

PE: -d^2 = 2 q.r - ||q||^2 - ||r||^2 as a K=16 bf16 matmul using the
split-bf16 trick (q ~ qh+ql, r ~ rh+rl, norms split hi/lo as well), so
products are exact bf16xbf16 accumulated in fp32 -> ~fp32 accuracy at
1 cycle/row (4x faster than the fp32 PE path). Stationary = 128 refs
per row tile, moving = 512 queries per matmul.

Consume per [128, 2048] PSUM group:
  1. copy+downcast PSUM -> fp16 SBUF tile (ScalarE mostly — it is
     otherwise idle — DVE for a tunable fraction),
  2. DVE reduce_max of the fp16 tile (fast 2-byte SBUF mode) chained
     into the per-ref RMS[:, r],
  3. elementwise fp16 max into the per-query accumulator CM (GPSIMD /
     DVE split, tunable).
Tail: clamp+convert CM -> fp32, PE-transpose 128x128 blocks into PSUM,
one 3D reduce_max over the candidate axis, sqrt, partial sums.

kernel(pred, label) takes the full inputs, shards on host (layout +
tiny O(N*D) augmentation only), runs the SPMD program on cores 0-7 via
run_bass_kernel_spmd, and combines the small per-core outputs.
"""

import os
import sys

import numpy as np

for _p in ("/opt/trn_rl_repo", "/root/.axon_site/_ro/trn_rl_repo"):
    if os.path.isdir(_p) and _p not in sys.path:
        sys.path.append(_p)

import ml_dtypes

import concourse.bacc as bacc
import concourse.mybir as mybir
from concourse import tile
from concourse.bass_utils import run_bass_kernel_spmd

F32 = mybir.dt.float32
F16 = mybir.dt.float16
BF16 = mybir.dt.bfloat16
NPBF16 = ml_dtypes.bfloat16
OP_MAX = mybir.AluOpType.max
AX_X = mybir.AxisListType.X
SQRT = mybir.ActivationFunctionType.Sqrt
COPY = mybir.ActivationFunctionType.Copy

B = 4
N = 8192
NCORES = 8
NEG16 = -60000.0

# full-size kernel geometry
NQ = N // 2      # queries per core (pred half)
NR = N           # refs per core (all labels of the batch)
MMN = 512        # moving free dim per matmul (one PSUM bank)
K = 16           # split-bf16 augmented contraction dim


def build_program(nq=NQ, nr=NR, mmn=MMN, dve_copy_every=5,
                  repeats=1, skip_fold=0, skip_red=0, skip_copy=0, scp_bufs=4,
                  use_bf16=1, loop_n=0, red_mode=1, gmm=4):
    """Emit + compile the per-core program.

    dve_copy_every: every n-th ref row-tile has its group-0 PSUM read done
      on DVE (as a fused tensor_scalar copy+reduce) instead of ACT, to
      balance the ACT copy load against DVE (0 = all copies on ACT).
    repeats: emit the whole compute body N times (benchmarking aid —
    wall-clock slope over repeats isolates device time from dispatch).
    """
    nchunk = gmm * mmn             # columns per consume group
    ngroup = nq // nchunk          # consume groups per ref row-tile
    rt = nr // 128                 # ref row-tiles
    psum_bufs = 8 // gmm           # PSUM slots (gmm banks each)
    assert nq % nchunk == 0 and nr % 128 == 0 and nq % 128 == 0

    S16 = BF16 if use_bf16 else F16
    nc = bacc.Bacc("TRN2", target_bir_lowering=False, debug=False)
    qs_d = nc.dram_tensor("qs", [128, nq // 4], BF16, kind="ExternalInput")
    rs_d = nc.dram_tensor("rs", [128, nr], BF16, kind="ExternalInput")
    id_d = nc.dram_tensor("ident", [128, 128], F32, kind="ExternalInput")
    refout_d = nc.dram_tensor("ref_out", [128, rt], F32, kind="ExternalOutput")
    qout_d = nc.dram_tensor("q_out", [128, 1], F32, kind="ExternalOutput")

    with tile.TileContext(nc) as tc:
        with (
            tc.tile_pool(name="const", bufs=1) as const,
            tc.tile_pool(name="rmp", bufs=2) as rmp,
            tc.tile_pool(name="scp", bufs=scp_bufs) as scp,
            tc.tile_pool(name="tail", bufs=1) as tail,
        ):
            RS = const.tile([128, nr], BF16)
            nc.sync.dma_start(RS[:], rs_d.ap())
            QS = const.tile([128, nq // 4], BF16)
            nc.sync.dma_start(QS[:], qs_d.ap())
            IDENT = const.tile([128, 128], F32)
            nc.sync.dma_start(IDENT[:], id_d.ap())
            CM = const.tile([128, nq], S16)
            RMS = const.tile([128, rt], F32)

            # body (repeated `repeats`/looped `loop_n` times for benchmarking)
            main_psum = tc.tile_pool(name="psum", bufs=psum_bufs, space="PSUM")
            psum = main_psum.__enter__()
            loop_ctx = tc.For_i(0, loop_n, 1) if loop_n >= 2 else None
            if loop_ctx is not None:
                loop_ctx.__enter__()
            for rep in range(repeats):
                if skip_copy:
                    skip_red = skip_fold = 1
                nc.vector.memset(CM[:], NEG16)
                if skip_red:
                    nc.vector.memset(RMS[:], NEG16)
                for r in range(rt):
                    # one contiguous fp16 copy target for the whole row tile
                    sc = scp.tile([128, nq], S16, tag="sc")
                    dve_rt = (ngroup > 1 and dve_copy_every
                              and r % dve_copy_every == 0)
                    rg0 = None
                    for j2 in range(ngroup):
                        ps = psum.tile([128, nchunk], F32)
                        for i in range(gmm):
                            nc.tensor.matmul(
                                ps[:, i * mmn:(i + 1) * mmn],
                                RS[32 * i:32 * i + K, r * 128:(r + 1) * 128],
                                QS[32 * i:32 * i + K, j2 * mmn:(j2 + 1) * mmn],
                                start=True,
                                stop=True,
                                tile_position=(32 * i, 0),
                            )
                        sc_sl = sc[:, j2 * nchunk:(j2 + 1) * nchunk]
                        if skip_copy:
                            pass
                        elif dve_rt and j2 == 0:
                            # DVE reads this PSUM group: fused copy+reduce
                            rg0 = rmp.tile([128, 1], F32, tag="rg0")
                            nc.vector.tensor_scalar(
                                sc_sl, ps[:], -3.0e38, None, OP_MAX, OP_MAX,
                                accum_out=rg0[:])
                        else:
                            nc.scalar.activation(sc_sl, ps[:], COPY)
                    # per-ref reduce over the whole row tile (fast fused
                    # tensor_scalar on 2-byte SBUF; out is a throwaway copy)
                    if skip_red:
                        pass
                    elif dve_rt:
                        rg1 = rmp.tile([128, 1], F32, tag="rg1")
                        if red_mode == 1:
                            nc.vector.reduce_max(rg1[:], sc[:, nchunk:],
                                                 axis=AX_X)
                        else:
                            scd = scp.tile([128, nq - nchunk], S16, tag="scd1")
                            nc.vector.tensor_scalar(
                                scd[:], sc[:, nchunk:], -3.0e38, None, OP_MAX,
                                OP_MAX, accum_out=rg1[:])
                        nc.vector.tensor_max(RMS[:, r:r + 1], rg0[:], rg1[:])
                    elif red_mode == 1:
                        nc.vector.reduce_max(RMS[:, r:r + 1], sc[:], axis=AX_X)
                    elif red_mode == 2:
                        t8 = rmp.tile([128, 8], F32, tag="t8")
                        nc.vector.max(t8[:], sc[:])
                        nc.vector.tensor_copy(RMS[:, r:r + 1], t8[:, 0:1])
                    else:
                        scd = scp.tile([128, nq], S16, tag="scd")
                        nc.vector.tensor_scalar(
                            scd[:], sc[:], -3.0e38, None, OP_MAX, OP_MAX,
                            accum_out=RMS[:, r:r + 1])
                    # per-query fold into CM (full row-tile width)
                    if not skip_fold:
                        nc.vector.tensor_max(CM[:], CM[:], sc[:])

                nc.sync.dma_start(refout_d.ap(), RMS[:])

                # per-query direction: max over the 128 partitions of CM.
                # Clamp+convert CM16 -> fp32, PE-transpose each 128x128 block
                # into PSUM (reusing the main 4-bank psum tiles), then a
                # reduce_max over the (now innermost-free) candidate axis,
                # then sqrt of the negated minima and sum.
                CM32 = tail.tile([128, nq], F32)
                nc.vector.tensor_scalar_min(CM32[:], CM[:], 0.0)
                nblk = nq // 128
                nbp = nchunk // 128    # transpose blocks per pass
                q2 = tail.tile([128, nblk], F32)
                for h2 in range(nq // nchunk):
                    pst = psum.tile([128, nchunk], F32, tag="ps")
                    for b in range(nbp):
                        blk = h2 * nbp + b
                        nc.tensor.transpose(
                            pst[:, b * 128:(b + 1) * 128],
                            CM32[:, blk * 128:(blk + 1) * 128],
                            IDENT[:],
                        )
                    nc.vector.tensor_reduce(
                        q2[:, h2 * nbp:(h2 + 1) * nbp],
                        pst[:].rearrange("p (b c) -> p b c", c=128),
                        axis=AX_X, op=OP_MAX,
                    )
                # q2 holds v = max(-d^2) clamped <= 0; sqrt(-v) = distance.
                sq = tail.tile([128, nblk], F32)
                nc.scalar.activation(sq[:], q2[:], SQRT, bias=0.0, scale=-1.0)
                qsum = tail.tile([128, 1], F32)
                nc.vector.reduce_sum(qsum[:], sq[:], axis=AX_X)
                nc.sync.dma_start(qout_d.ap(), qsum[:])
            if loop_ctx is not None:
                loop_ctx.__exit__(None, None, None)
            main_psum.__exit__(None, None, None)

    nc.compile()
    return nc


def _split2(x):
    """fp32 -> (hi, lo) fp32 arrays exactly representable in bf16."""
    hi = x.astype(NPBF16).astype(np.float32)
    lo = (x - hi).astype(NPBF16).astype(np.float32)
    return hi, lo


def prep_core(q, r, mmn=MMN):
    """Build the split-bf16 augmented layouts for one core.

    dot(qaug[:, n], raug[:, m]) = 2 qt.rt - ||qt||^2 - ||rt||^2
                                = -||qt - rt||^2
    with qt = qh+ql (~fp32 accurate), rt = rh+rl.
    """
    nq, nr = q.shape[0], r.shape[0]
    q = np.ascontiguousarray(q, np.float32)
    r = np.ascontiguousarray(r, np.float32)
    qh, ql = _split2(q)
    rh, rl = _split2(r)
    qt = qh + ql
    rtt = rh + rl
    q2h, q2l = _split2((qt * qt).sum(1, dtype=np.float32))
    r2h, r2l = _split2((rtt * rtt).sum(1, dtype=np.float32))
    ones_q = np.ones(nq, np.float32)
    ones_r = np.ones(nr, np.float32)
    qaug = np.concatenate([
        qh.T, ql.T, qh.T, ql.T,
        ones_q[None], ones_q[None], q2h[None], q2l[None],
    ])  # [16, nq]
    raug = np.concatenate([
        2 * rh.T, 2 * rh.T, 2 * rl.T, 2 * rl.T,
        -r2h[None], -r2l[None], -ones_r[None], -ones_r[None],
    ])  # [16, nr]
    # strip layouts for the 4-way tile_position packing:
    #   qs[32s+k, j2*mmn + m] = qaug[k, (4*j2 + s)*mmn + m]
    #   rs[32s+k, :]          = raug[k, :]   (replicated per strip)
    qv = qaug.reshape(K, nq // (4 * mmn), 4, mmn)
    qs = np.zeros((128, nq // 4), np.float32)
    rs = np.zeros((128, nr), np.float32)
    for s in range(4):
        qs[32 * s:32 * s + K, :] = qv[:, :, s, :].reshape(K, -1)
        rs[32 * s:32 * s + K, :] = raug
    return {
        "qs": qs.astype(NPBF16),
        "rs": rs.astype(NPBF16),
        "ident": np.eye(128, dtype=np.float32),
    }


def make_in_maps(pred, label):
    pred = np.asarray(pred, np.float32)
    label = np.asarray(label, np.float32)
    in_maps = []
    for b in range(B):
        for h in range(2):
            in_maps.append(prep_core(pred[b, h * NQ:(h + 1) * NQ], label[b]))
    return in_maps


def postprocess(results):
    # pred -> label direction: per-core device sums of sqrt(min d^2)
    sq_sum = sum(float(res["q_out"].sum(dtype=np.float64)) for res in results)
    # label -> pred: combine the two pred-halves per batch, then sqrt/sum
    ref_sum = 0.0
    for b in range(B):
        m = np.maximum(results[2 * b]["ref_out"], results[2 * b + 1]["ref_out"])
        ref_sum += float(np.sqrt(np.maximum(-m, 0.0)).sum(dtype=np.float64))
    return np.float32((sq_sum + ref_sum) / (B * N))


_PROGRAM = None


def _get_program():
    global _PROGRAM
    if _PROGRAM is None:
        _PROGRAM = build_program()
    return _PROGRAM


def run_on_hw(pred, label, trace=False):
    nc = _get_program()
    res = run_bass_kernel_spmd(nc, make_in_maps(pred, label),
                               list(range(NCORES)), trace=trace)
    return postprocess(res.results), res


def kernel(pred, label):
    out, _ = run_on_hw(pred, label)
    return out



# revision 6
# speedup vs baseline: 8.4571x; 8.4571x over previous
"""Chamfer loss kernel for Trainium2 (Bass/Tile), 8 NeuronCores.

Math: for each batch b, D_b[n, m] = ||pred[b,n] - label[b,m]||.
result = mean_n(min_m D) + mean_m(min_n D).

Sharding: 8 cores = 4 batches x 2 halves of the pred axis. Core c
(b = c//2, h = c%2) owns queries q = pred[b, h*4096:(h+1)*4096] (NQ=4096)
and all refs r = label[b] (NR=8192). One pass over the 4096 x 8192 block
of -d^2 produces BOTH reductions (per-ref partial min_n, per-query
complete min_m).

PE: -d^2 = 2 q.r - ||q||^2 - ||r||^2 as a K=16 bf16 matmul using the
split-bf16 trick, 4-way tile_position packing (4 PE quadrants work on 4
different query chunks concurrently).

End-to-end latency is dominated by the per-call axon dispatch (~80ms
fixed) plus input bytes through the tunnel (~15ms/MB). So:
  - each core receives ONE raw fp32 tensor x=[12288,3] (its 4096 pred
    points + 8192 label points, 147KB); the whole bf16-split/norms/strip
    augmentation runs on device.
  - outputs are packed into ONE [128, 65] f32 tensor per core.
  - the jitted shard_map callable is built once and cached (the stock
    run_bass_kernel_spmd path re-traces every call, ~500ms).
"""

import os
import sys

import numpy as np

for _p in ("/opt/trn_rl_repo", "/root/.axon_site/_ro/trn_rl_repo"):
    if os.path.isdir(_p) and _p not in sys.path:
        sys.path.append(_p)

import concourse.bacc as bacc
import concourse.mybir as mybir
from concourse import tile

F32 = mybir.dt.float32
BF16 = mybir.dt.bfloat16
I32 = mybir.dt.int32
OP_MAX = mybir.AluOpType.max
OP_EQ = mybir.AluOpType.is_equal
AX_X = mybir.AxisListType.X
SQRT = mybir.ActivationFunctionType.Sqrt
COPY = mybir.ActivationFunctionType.Copy

B = 4
N = 8192
NCORES = 8
NEG16 = -60000.0

NQ = N // 2      # queries per core (pred half)
NR = N           # refs per core (all labels of the batch)
NTOT = NQ + NR   # rows of the per-core input slab
MMN = 512        # moving free dim per matmul (one PSUM bank)
K = 16           # split-bf16 augmented contraction dim
RT = NR // 128   # ref row-tiles
CH = 2048        # prep chunk width


def emit_prep(nc, tc, prep, QS, RS, IDENT):
    """On-device input prep: from x=[NTOT,3] fp32 in DRAM build the
    augmented bf16 strip layouts.

    Strip rows (k within each 32-partition strip s):
      QS: 0-2 = 2*qh, 3-5 = 2*ql, 6-8 = 2*qh, 9-11 = 2*ql,
          12-13 = -1, 14 = q2h, 15 = q2l
      RS: 0-5 = rh (x2), 6-11 = rl (x2), 12 = r2h, 13 = r2l, 14-15 = -1
    so dot(qs_strip, rs_strip) = 2 qt.rt - q2 - r2 = -d^2, with
    qt = qh+ql (fp32-accurate), q2 = ||qt||^2 split hi/lo.
    QS column packing: strip s, dst col j2*512+c <- query (4*j2+s)*512+c.
    """
    x_d = nc.x_d
    # identity for the tail PE transpose: (col idx == partition idx)
    IP = prep.tile([128, 128], I32, tag="ip")
    nc.gpsimd.iota(IP[:], pattern=[[0, 128]], base=0, channel_multiplier=1)
    IC = prep.tile([128, 128], I32, tag="ic")
    nc.gpsimd.iota(IC[:], pattern=[[1, 128]], base=0, channel_multiplier=0)
    nc.vector.tensor_tensor(IDENT[:], IP[:], IC[:], OP_EQ)

    # compute-engine APs must start at partition 0/32/64/96; stage the -1
    # rows at partition 0 and DMA them into the strips.
    nq4 = QS.shape[1]
    nr = RS.shape[1]
    NEG1Q = prep.tile([2, nq4], BF16, tag="neg1q")
    nc.vector.memset(NEG1Q[:], -1.0)
    NEG1R = prep.tile([2, nr], BF16, tag="neg1r")
    nc.vector.memset(NEG1R[:], -1.0)
    ONES3 = prep.tile([3, 1], F32, tag="ones3")
    nc.vector.memset(ONES3[:], 1.0)
    for s in range(4):
        nc.sync.dma_start(QS[32 * s + 12:32 * s + 14, :], NEG1Q[:])
        nc.sync.dma_start(RS[32 * s + 14:32 * s + 16, :], NEG1R[:])

    pp = tc.tile_pool(name="prep_psum", bufs=2, space="PSUM")
    ppsum = pp.__enter__()
    for c in range(NTOT // CH):
        # transposed load: [CH, 3] rows -> [3, CH]
        T3 = prep.tile([3, CH], F32, tag="t3")
        nc.sync.dma_start(
            T3[:], x_d.ap()[c * CH:(c + 1) * CH, :].rearrange("n d -> d n"))
        # bf16 split: hi = bf16(x), lo = bf16(x - f32(hi))
        HB = prep.tile([3, CH], BF16, tag="hb")
        nc.scalar.activation(HB[:], T3[:], COPY)
        H32 = prep.tile([3, CH], F32, tag="h32")
        nc.vector.tensor_copy(H32[:], HB[:])
        L32 = prep.tile([3, CH], F32, tag="l32")
        nc.vector.tensor_sub(L32[:], T3[:], H32[:])
        LB = prep.tile([3, CH], BF16, tag="lb")
        nc.scalar.activation(LB[:], L32[:], COPY)
        LB32 = prep.tile([3, CH], F32, tag="lb32")
        nc.gpsimd.tensor_copy(LB32[:], LB[:])
        # norms of qt = f32(hi) + f32(lo), summed across the 3 partitions
        QT = prep.tile([3, CH], F32, tag="qt")
        nc.vector.tensor_add(QT[:], H32[:], LB32[:])
        SQ = prep.tile([3, CH], F32, tag="sq")
        nc.vector.tensor_mul(SQ[:], QT[:], QT[:])
        # sum across the 3 coordinate partitions via a [3,1] ones-matmul
        N2 = prep.tile([1, CH], F32, tag="n2")
        for cc in range(CH // 512):
            ps = ppsum.tile([1, 512], F32, tag="pnorm")
            nc.tensor.matmul(ps[:], ONES3[:], SQ[:, cc * 512:(cc + 1) * 512],
                             start=True, stop=True)
            nc.scalar.activation(N2[:, cc * 512:(cc + 1) * 512], ps[:], COPY)
        # split the norms hi/lo as well
        N2H = prep.tile([1, CH], BF16, tag="n2h")
        nc.scalar.activation(N2H[:], N2[:], COPY)
        N2H32 = prep.tile([1, CH], F32, tag="n2h32")
        nc.gpsimd.tensor_copy(N2H32[:], N2H[:])
        N2L32 = prep.tile([1, CH], F32, tag="n2l32")
        nc.vector.tensor_sub(N2L32[:], N2[:], N2H32[:])
        N2L = prep.tile([1, CH], BF16, tag="n2l")
        nc.scalar.activation(N2L[:], N2L32[:], COPY)

        if c * CH < NQ:
            # query chunk j2 = c: scale by 2 (exact in bf16), distribute
            # 512-col blocks to the 4 strips
            j2 = c
            QH2 = prep.tile([3, CH], BF16, tag="qh2")
            nc.vector.tensor_scalar_mul(QH2[:], HB[:], 2.0)
            QL2 = prep.tile([3, CH], BF16, tag="ql2")
            nc.vector.tensor_scalar_mul(QL2[:], LB[:], 2.0)
            for s in range(4):
                bs = slice(s * MMN, (s + 1) * MMN)
                dst = slice(j2 * MMN, (j2 + 1) * MMN)
                r0 = 32 * s
                nc.sync.dma_start(QS[r0 + 0:r0 + 3, dst], QH2[:, bs])
                nc.sync.dma_start(QS[r0 + 3:r0 + 6, dst], QL2[:, bs])
                nc.sync.dma_start(QS[r0 + 6:r0 + 9, dst], QH2[:, bs])
                nc.sync.dma_start(QS[r0 + 9:r0 + 12, dst], QL2[:, bs])
                nc.sync.dma_start(QS[r0 + 14:r0 + 15, dst], N2H[:, bs])
                nc.sync.dma_start(QS[r0 + 15:r0 + 16, dst], N2L[:, bs])
        else:
            rc = c * CH - NQ
            dst = slice(rc, rc + CH)
            for s in range(4):
                r0 = 32 * s
                nc.sync.dma_start(RS[r0 + 0:r0 + 3, dst], HB[:])
                nc.sync.dma_start(RS[r0 + 3:r0 + 6, dst], HB[:])
                nc.sync.dma_start(RS[r0 + 6:r0 + 9, dst], LB[:])
                nc.sync.dma_start(RS[r0 + 9:r0 + 12, dst], LB[:])
                nc.sync.dma_start(RS[r0 + 12:r0 + 13, dst], N2H[:])
                nc.sync.dma_start(RS[r0 + 13:r0 + 14, dst], N2L[:])
    pp.__exit__(None, None, None)


def build_program(nq=NQ, nr=NR, mmn=MMN, dve_copy_every=5, scp_bufs=4,
                  gmm=4, debug_dump=False):
    """Emit + compile the per-core program."""
    nchunk = gmm * mmn             # columns per consume group
    ngroup = nq // nchunk          # consume groups per ref row-tile
    rt = nr // 128                 # ref row-tiles
    psum_bufs = 8 // gmm           # PSUM slots (gmm banks each)
    assert nq % nchunk == 0 and nr % 128 == 0

    nc = bacc.Bacc("TRN2", target_bir_lowering=False, debug=False)
    nc.x_d = nc.dram_tensor("x", [NTOT, 3], F32, kind="ExternalInput")
    out_d = nc.dram_tensor("out", [128, rt + 1], F32, kind="ExternalOutput")
    if debug_dump:
        qs_d = nc.dram_tensor("qs_dbg", [128, nq // 4], BF16,
                              kind="ExternalOutput")
        rs_d = nc.dram_tensor("rs_dbg", [128, nr], BF16,
                              kind="ExternalOutput")
        id_d = nc.dram_tensor("id_dbg", [128, 128], F32,
                              kind="ExternalOutput")

    with tile.TileContext(nc) as tc:
        with (
            tc.tile_pool(name="const", bufs=1) as const,
            tc.tile_pool(name="rmp", bufs=2) as rmp,
            tc.tile_pool(name="scp", bufs=scp_bufs) as scp,
            tc.tile_pool(name="tail", bufs=1) as tail,
        ):
            QS = const.tile([128, nq // 4], BF16)
            RS = const.tile([128, nr], BF16)
            IDENT = const.tile([128, 128], F32)
            CM = const.tile([128, nq], BF16)
            RMS = const.tile([128, rt], F32)

            with tc.tile_pool(name="prep", bufs=1) as prep:
                emit_prep(nc, tc, prep, QS, RS, IDENT)
            if debug_dump:
                nc.sync.dma_start(qs_d.ap(), QS[:])
                nc.sync.dma_start(rs_d.ap(), RS[:])
                nc.sync.dma_start(id_d.ap(), IDENT[:])

            with tc.tile_pool(name="psum", bufs=psum_bufs,
                              space="PSUM") as psum:
                nc.vector.memset(CM[:], NEG16)
                nc.vector.memset(RMS[:], NEG16)
                for r in range(rt):
                    # one contiguous bf16 copy target for the row tile
                    sc = scp.tile([128, nq], BF16, tag="sc")
                    dve_rt = (ngroup > 1 and dve_copy_every
                              and r % dve_copy_every == 0)
                    rg0 = None
                    for j2 in range(ngroup):
                        ps = psum.tile([128, nchunk], F32)
                        for i in range(gmm):
                            nc.tensor.matmul(
                                ps[:, i * mmn:(i + 1) * mmn],
                                RS[32 * i:32 * i + K, r * 128:(r + 1) * 128],
                                QS[32 * i:32 * i + K, j2 * mmn:(j2 + 1) * mmn],
                                start=True,
                                stop=True,
                                tile_position=(32 * i, 0),
                            )
                        sc_sl = sc[:, j2 * nchunk:(j2 + 1) * nchunk]
                        if dve_rt and j2 == 0:
                            # DVE reads this PSUM group: fused copy+reduce
                            rg0 = rmp.tile([128, 1], F32, tag="rg0")
                            nc.vector.tensor_scalar(
                                sc_sl, ps[:], -3.0e38, None, OP_MAX, OP_MAX,
                                accum_out=rg0[:])
                        else:
                            nc.scalar.activation(sc_sl, ps[:], COPY)
                    # per-ref reduce over the whole row tile
                    if dve_rt:
                        rg1 = rmp.tile([128, 1], F32, tag="rg1")
                        nc.vector.reduce_max(rg1[:], sc[:, nchunk:], axis=AX_X)
                        nc.vector.tensor_max(RMS[:, r:r + 1], rg0[:], rg1[:])
                    else:
                        nc.vector.reduce_max(RMS[:, r:r + 1], sc[:], axis=AX_X)
                    # per-query fold into CM (full row-tile width)
                    nc.vector.tensor_max(CM[:], CM[:], sc[:])

                nc.sync.dma_start(out_d.ap()[:, 0:rt], RMS[:])

                # per-query direction: max over the 128 partitions of CM.
                CM32 = tail.tile([128, nq], F32)
                nc.vector.tensor_scalar_min(CM32[:], CM[:], 0.0)
                nblk = nq // 128
                nbp = nchunk // 128    # transpose blocks per pass
                q2 = tail.tile([128, nblk], F32)
                for h2 in range(nq // nchunk):
                    pst = psum.tile([128, nchunk], F32, tag="ps")
                    for b in range(nbp):
                        blk = h2 * nbp + b
                        nc.tensor.transpose(
                            pst[:, b * 128:(b + 1) * 128],
                            CM32[:, blk * 128:(blk + 1) * 128],
                            IDENT[:],
                        )
                    nc.vector.tensor_reduce(
                        q2[:, h2 * nbp:(h2 + 1) * nbp],
                        pst[:].rearrange("p (b c) -> p b c", c=128),
                        axis=AX_X, op=OP_MAX,
                    )
                # q2 holds v = max(-d^2) clamped <= 0; sqrt(-v) = distance.
                sq = tail.tile([128, nblk], F32)
                nc.scalar.activation(sq[:], q2[:], SQRT, bias=0.0, scale=-1.0)
                qsum = tail.tile([128, 1], F32)
                nc.vector.reduce_sum(qsum[:], sq[:], axis=AX_X)
                nc.sync.dma_start(out_d.ap()[:, rt:rt + 1], qsum[:])

    nc.compile()
    return nc


def make_slab(pred, label):
    """Concatenated per-core input: core c=(b,h) gets rows
    [pred[b, h*NQ:(h+1)*NQ]; label[b]] as [NTOT, 3] fp32."""
    pred = np.asarray(pred, np.float32)
    label = np.asarray(label, np.float32)
    X = np.empty((NCORES * NTOT, 3), np.float32)
    for c in range(NCORES):
        b, h = divmod(c, 2)
        o = c * NTOT
        X[o:o + NQ] = pred[b, h * NQ:(h + 1) * NQ]
        X[o + NQ:o + NTOT] = label[b]
    return X


def postprocess(outs):
    """outs: [NCORES, 128, RT+1] f32."""
    sq_sum = float(outs[:, :, RT].sum(dtype=np.float64))
    ref_sum = 0.0
    for b in range(B):
        m = np.maximum(outs[2 * b, :, :RT], outs[2 * b + 1, :, :RT])
        ref_sum += float(np.sqrt(np.maximum(-m, 0.0)).sum(dtype=np.float64))
    return np.float32((sq_sum + ref_sum) / (B * N))


_PROGRAM = None
_SHARDED = None


def _get_program():
    global _PROGRAM
    if _PROGRAM is None:
        _PROGRAM = build_program()
    return _PROGRAM


def _get_sharded():
    """Build the jitted 8-core shard_map callable ONCE (the stock
    run_bass_kernel_spmd re-creates it per call, paying ~0.5s retrace)."""
    global _SHARDED
    if _SHARDED is None:
        import jax
        from jax.sharding import Mesh, PartitionSpec
        from jax.experimental.shard_map import shard_map
        from concourse.bass2jax import (_bass_exec_p, partition_id_tensor,
                                        install_neuronx_cc_hook)
        install_neuronx_cc_hook()
        nc = _get_program()
        partition_name = (nc.partition_id_tensor.name
                          if nc.partition_id_tensor else None)
        out_avals = (jax.core.ShapedArray((128, RT + 1), np.float32),)
        in_names = ("x", "out") + ((partition_name,) if partition_name else ())

        def _body(x, zout):
            operands = [x, zout]
            if partition_name is not None:
                operands.append(partition_id_tensor())
            outs = _bass_exec_p.bind(
                *operands, out_avals=out_avals, in_names=in_names,
                out_names=("out",), lowering_input_output_aliases=(),
                sim_require_finite=True, sim_require_nnan=True, nc=nc)
            return tuple(outs)

        devices = jax.devices()[:NCORES]
        mesh = Mesh(np.asarray(devices), ("core",))
        _SHARDED = jax.jit(
            shard_map(_body, mesh=mesh,
                      in_specs=(PartitionSpec("core"),) * 2,
                      out_specs=(PartitionSpec("core"),), check_rep=False),
            donate_argnums=(1,), keep_unused=True)
    return _SHARDED


def run_on_hw(pred, label, trace=False):
    """Returns (result, res-like object). Fast path: cached jit callable.
    trace=True falls back to the stock (slower, profiled) path."""
    from concourse.bass_utils import run_bass_kernel_spmd, axon_active

    X = make_slab(pred, label)
    if trace or not axon_active():
        nc = _get_program()
        in_maps = [{"x": X[c * NTOT:(c + 1) * NTOT]} for c in range(NCORES)]
        res = run_bass_kernel_spmd(nc, in_maps, list(range(NCORES)),
                                   trace=trace)
        outs = np.stack([r["out"] for r in res.results])
        return postprocess(outs), res

    sharded = _get_sharded()
    zeros = np.zeros((NCORES * 128, RT + 1), np.float32)
    (out,) = sharded(X, zeros)
    outs = np.asarray(out).reshape(NCORES, 128, RT + 1)

    class _Res:
        results = None
        exec_time_ns = None
        profile_json = None
    return postprocess(outs), _Res()


def kernel(pred, label):
    out, _ = run_on_hw(pred, label)
    return out
